# revision 42
# baseline (speedup 1.0000x reference)
"""Trainium2 Bass kernel for softmax RGB blend (pytorch3d NoLightShader).

Full inputs (N=8, H=512, W=512, K=8) are sharded batch-wise across 8
NeuronCores (one image per core); the blend is per-pixel, no cross-core
communication.

Host-side input encoding (per core):
    mask folded into the data (pix_to_face never shipped):
        d_eff = where(mask, dists, 1.0)        -> sigmoid(-d/SIGMA) = 0
        z_inv = (ZFAR - zbuf)/(ZFAR - ZNEAR) * mask
    z shipped as uint16 fixed point (z16 = round(65535 * z_inv)): u16 order
    matches float order, so the K-max runs in u16, and ACT's free affine
    (scale/bias) turns u16 straight into exp arguments.
    dists/colors shipped as bf16. Per-tile layout is k-major [K, T] (colors
    [3, K, T]) so every K-reduction is a contiguous pairwise fold tree at
    DVE 2x bf16 mode (tensor_reduce is stuck at 1x). dists ship as one
    up-front stream so ALL sigmoids run in a prepass -- the sigmoid and
    ln/exp ACT table sets otherwise swap twice per tile (~2.7us a load).
    Output is planar bf16 [4, T] per tile (r|g|b|a), host transposes.

Math per pixel:  p_k = sigmoid(-d_k/SIGMA); q_k = 1-p_k
    alpha = 1 - prod_k q_k     (DVE computes mq=p-1; GPSIMD mult fold tree;
                                8 negations cancel)
    zmax  = max_k z_k          (DVE u16 max fold tree)
    w_k   = p_k * exp((z_k - zmax)/GAMMA)  (zd=zmax-z fp16 on GPSIMD, exp ACT)
    delta = exp((EPS - zmax)/GAMMA)
    denom = sum_k w_k + delta              (DVE bf16 add fold tree)
    rgb   = (sum_k w_k c_k + delta)/denom  (bg=1; wc + fold tree on DVE)
    out   = [rgb, alpha]

Engines: SP HWDGE DMAs (d-stream + 1 in + 1 out per tile) | ACT: sigmoid
prepass, exp(zd), delta, ln(denom), rcp=exp(-ln), alpha | DVE: zmax folds,
mq, w, wc, wsum folds, denom, csum folds, t3, rgb | GPSIMD: zd, prod-q
folds. Raw bass, two-pass mark/wait scheduling, double-buffered tiles.
"""

import sys
from contextlib import ExitStack

import numpy as np

if "/opt/trn_rl_repo" not in sys.path:
    sys.path.insert(0, "/opt/trn_rl_repo")

SIGMA = 1e-4
GAMMA = 1e-4
ZNEAR = 1.0
ZFAR = 100.0
EPS = 1e-10

P = 128
K = 8
N_CORES = 8
ROWS = 2048          # H*W / P
T = 256              # pixels per partition per tile
NT = ROWS // T       # 8 tiles
TK = T * K           # 2048
IN_W = TK + TK * 3        # u16 words per tile: z | col
OUT_W = T * 4             # bf16 words per tile (planar r|g|b|a)

S16G = (1.0 / 65535.0) / GAMMA   # u16 step -> 1/GAMMA units


def build_program():
    import concourse.bass as bass
    from concourse import mybir

    dt = mybir.dt
    f32 = dt.float32
    bf16 = dt.bfloat16
    fp16 = dt.float16
    u16 = dt.uint16
    Alu = mybir.AluOpType
    Act = mybir.ActivationFunctionType

    n = NT

    nc = bass.Bass()

    in_d = nc.dram_tensor("inb", [P, n * IN_W], u16, kind="ExternalInput")
    d_d = nc.dram_tensor("din", [P, n * TK], u16, kind="ExternalInput")
    out_d = nc.dram_tensor("out", [P, n * OUT_W], u16, kind="ExternalOutput")

    # const AP for the delta bias (EPS/GAMMA); framework pre-registers 0.0/1.0.
    # Written by the first DVE op; every ACT reader (delta) transitively waits
    # on later DVE marks, so no barrier is needed.
    cbias = nc.alloc_sbuf_tensor("c_epsg", [P, 1], f32)
    nc.const_aps.aps[(f32, EPS / GAMMA)] = cbias.ap()

    with ExitStack() as ctx:
        def sb(name, w, dty=bf16):
            return ctx.enter_context(nc.sbuf_tensor(name, [P, w], dty))

        NB = 3  # input tile buffers
        inb = [sb(f"inb{j}", IN_W, u16) for j in range(NB)]
        d_sb = sb("dall", n * TK, u16)   # d bf16; sigmoid overwrites in place
        ot = [sb(f"ot{j}", OUT_W, u16) for j in range(2)]

        q_b = [sb(f"q{j}", TK) for j in range(2)]
        ex_b = [sb(f"ex{j}", TK) for j in range(2)]
        zd_b = [sb(f"zd{j}", TK, fp16) for j in range(2)]
        zmax = [sb(f"zmax{j}", T, u16) for j in range(2)]
        delta = [sb(f"delta{j}", T) for j in range(2)]
        qsum = [sb(f"qsum{j}", T, f32) for j in range(2)]
        pqt = sb("pqt", T, f32)
        rcp = [sb(f"rcp{j}", T) for j in range(2)]
        t3b = [sb(f"t3{j}", T * 3) for j in range(2)]
        denom = [sb(f"denom{j}", T, f32) for j in range(2)]

        zm4 = sb("zm4", TK // 2, u16)
        zm2 = sb("zm2", TK // 4, u16)
        w_b = sb("w", TK)
        ws4 = sb("ws4", TK // 2)
        ws2 = sb("ws2", TK // 4)
        wsum = sb("wsum", T)
        q4 = sb("q4", TK // 2)
        q2 = sb("q2", TK // 4)
        wc = sb("wc", TK * 3)
        cs4 = sb("cs4", TK * 3 // 2)
        cs2 = sb("cs2", TK * 3 // 4)
        csum = sb("csum", T * 3)
        lnden = sb("lnden", T, f32)

        s_in = [
            ctx.enter_context(nc.semaphore("s_in0")),
            ctx.enter_context(nc.semaphore("s_in1")),
            ctx.enter_context(nc.semaphore("s_in2")),
        ]
        s_out = [
            ctx.enter_context(nc.semaphore("s_out0")),
            ctx.enter_context(nc.semaphore("s_out1")),
        ]
        s_ind = [
            ctx.enter_context(nc.semaphore(f"s_ind{j}")) for j in range(4)
        ]
        s_act = ctx.enter_context(nc.semaphore("s_act"))
        s_dve = ctx.enter_context(nc.semaphore("s_dve"))
        s_gp = ctx.enter_context(nc.semaphore("s_gp"))

        marks = {}

        def mk(engkey, name, t, ctr):
            marks[(engkey, name, t)] = ctr

        # ---- SBUF views -------------------------------------------------
        def z_kt(j):      # [P, K, T] u16
            return inb[j][:, 0:TK].rearrange("p (k t) -> p k t", k=K)

        def col_ckt(j):   # [P, 3, K, T] bf16
            return inb[j][:, TK:IN_W].bitcast(bf16).rearrange(
                "p (c k t) -> p c k t", c=3, k=K
            )

        def d_bf(i):      # [P, TK] bf16, tile i of the d stream
            return d_sb[:, bass.ts(i, TK)].bitcast(bf16)

        def p_t(i):       # [P, TK] bf16, tile i of sigmoid (in-place over d)
            return d_sb[:, bass.ts(i, TK)].bitcast(bf16)

        def ot_rgb(j):    # [P, 3, T] bf16 planar
            return ot[j][:, 0:3 * T].bitcast(bf16).rearrange(
                "p (c t) -> p c t", c=3
            )

        def ot_a(j):      # [P, T] bf16
            return ot[j][:, 3 * T:4 * T].bitcast(bf16)

        # ---- schedules --------------------------------------------------
        def sched_sp(sp):
            if sp is not None:
                # interleave the first input tiles with d quarters; each d
                # quarter gets its own FULL-value sem so prepass chunks can
                # start as soon as their d lands
                q = n * TK // 4
                sp.dma_start(
                    out=inb[0][:], in_=in_d[:, bass.ts(0, IN_W)]
                ).then_inc(s_in[0], 16)
                sp.dma_start(out=d_sb[:, 0:q], in_=d_d[:, 0:q]
                             ).then_inc(s_ind[0], 16)
                sp.dma_start(
                    out=inb[1][:], in_=in_d[:, bass.ts(1, IN_W)]
                ).then_inc(s_in[1], 16)
                sp.dma_start(out=d_sb[:, q:2 * q], in_=d_d[:, q:2 * q]
                             ).then_inc(s_ind[1], 16)
                sp.dma_start(
                    out=inb[2][:], in_=in_d[:, bass.ts(2, IN_W)]
                ).then_inc(s_in[2], 16)
                sp.dma_start(out=d_sb[:, 2 * q:3 * q], in_=d_d[:, 2 * q:3 * q]
                             ).then_inc(s_ind[2], 16)
                sp.dma_start(out=d_sb[:, 3 * q:4 * q], in_=d_d[:, 3 * q:4 * q]
                             ).then_inc(s_ind[3], 16)
            for i in range(NB, n):
                j = i % NB
                if sp is not None:
                    sp.wait_ge(s_dve, marks[("d", "wc", i - NB)])
                    sp.dma_start(
                        out=inb[j][:], in_=in_d[:, bass.ts(i, IN_W)]
                    ).then_inc(s_in[j], 16)
            if sp is not None:
                sp.wait_ge(s_out[0], 16 * ((n + 1) // 2))
                sp.wait_ge(s_out[1], 16 * (n // 2))

        def emit_ex(act, t):
            act.activation(ex_b[t % 2][:], zd_b[t % 2][:], Act.Exp,
                           scale=S16G).then_inc(s_act, 1)

        def emit_delta(act, t):
            act.activation(
                delta[t % 2][:], zmax[t % 2][:], Act.Exp,
                bias=EPS / GAMMA, scale=-S16G,
            ).then_inc(s_act, 1)

        def emit_lnq(act, t):
            act.activation(q_b[t % 2][:], p_t(t), Act.Ln,
                           bias=1.0, scale=-1.0).then_inc(s_act, 1)

        def sched_act(act):
            c = 0
            # sigmoid prepass in 2-tile chunks (one table set); tile 0's
            # exp/ln ops are interleaved after chunk 0 so DVE's w(0) isn't
            # blocked on the whole prepass
            for ch in range(n // 2):
                if act is not None:
                    act.wait_ge(s_ind[ch], 16)
                    act.activation(
                        d_sb[:, bass.ts(ch, 2 * TK)].bitcast(bf16),
                        d_sb[:, bass.ts(ch, 2 * TK)].bitcast(bf16),
                        Act.Sigmoid, scale=-1.0 / SIGMA,
                    ).then_inc(s_act, 1)
                c += 1
                mk("a", "p", 2 * ch, c)
                mk("a", "p", 2 * ch + 1, c)
                if ch == 0:
                    if act is not None:
                        act.wait_ge(s_dve, marks[("d", "zd", 0)])
                        emit_ex(act, 0)
                    c += 1; mk("a", "ex", 0, c)
                    if act is not None:
                        emit_delta(act, 0)
                    c += 1; mk("a", "delta", 0, c)
                    if act is not None:
                        emit_lnq(act, 0)
                    c += 1; mk("a", "lnq", 0, c)
            for i in range(n + 2):
                t = i - 1
                u = i - 2
                if 1 <= t < n:
                    if act is not None:
                        act.wait_ge(s_dve, marks[("d", "zd", t)])
                        if t >= 2:
                            act.wait_ge(s_dve, marks[("d", "w", t - 2)])
                        emit_ex(act, t)
                    c += 1; mk("a", "ex", t, c)
                    if act is not None:
                        if t >= 2:
                            act.wait_ge(s_dve, marks[("d", "t3", t - 2)])
                        emit_delta(act, t)
                    c += 1; mk("a", "delta", t, c)
                    if act is not None:
                        emit_lnq(act, t)
                    c += 1; mk("a", "lnq", t, c)
                if u >= 0:
                    if act is not None:
                        act.wait_ge(s_dve, marks[("d", "denom", u)])
                        act.activation(lnden[:], denom[u % 2][:], Act.Ln
                                       ).then_inc(s_act, 1)
                    c += 1; mk("a", "lnd", u, c)
                    if act is not None:
                        if u >= 2:
                            act.wait_ge(s_dve, marks[("d", "rgb", u - 2)])
                        act.activation(rcp[u % 2][:], lnden[:], Act.Exp,
                                       scale=-1.0).then_inc(s_act, 1)
                    c += 1; mk("a", "rcp", u, c)
                    if act is not None:
                        act.wait_ge(s_dve, marks[("d", "qsum", u)])
                        act.activation(pqt[:], qsum[u % 2][:], Act.Exp
                                       ).then_inc(s_act, 1)
                    c += 1; mk("a", "pq", u, c)
                    if act is not None:
                        if u >= 2:
                            act.wait_ge(s_out[u % 2], 16 * (u // 2))
                        act.activation(ot_a(u % 2), pqt[:], Act.Copy,
                                       bias=1.0, scale=-1.0).then_inc(s_act, 1)
                    c += 1; mk("a", "alpha", u, c)
                    if act is not None:
                        act.wait_ge(s_dve, marks[("d", "rgb", u)])
                        act.dma_start(
                            out=out_d[:, bass.ts(u, OUT_W)], in_=ot[u % 2][:]
                        ).then_inc(s_out[u % 2], 16)

        def sched_dve(dve):
            c = 0
            if dve is not None:
                dve.memset(cbias.ap(), EPS / GAMMA)
            for i in range(n + 2):
                t = i - 1
                u = i - 2
                if i < n:
                    j = i % 2
                    jb = i % NB
                    if dve is not None:
                        dve.wait_ge(s_in[jb], 16 * (i // NB + 1))
                        if i >= 2:
                            dve.wait_ge(s_act, marks[("a", "delta", i - 2)])
                        zv = inb[jb][:, 0:TK]
                        dve.tensor_tensor(
                            out=zm4[:], in0=zv[:, 0:TK // 2],
                            in1=zv[:, TK // 2:TK], op=Alu.max,
                        ).then_inc(s_dve, 1)
                    c += 1; mk("d", "zm1", i, c)
                    if dve is not None:
                        dve.tensor_tensor(
                            out=zm2[:], in0=zm4[:, 0:TK // 4],
                            in1=zm4[:, TK // 4:TK // 2], op=Alu.max,
                        ).then_inc(s_dve, 1)
                    c += 1; mk("d", "zm2", i, c)
                    if dve is not None:
                        dve.tensor_tensor(
                            out=zmax[j][:], in0=zm2[:, 0:T],
                            in1=zm2[:, T:2 * T], op=Alu.max,
                        ).then_inc(s_dve, 1)
                    c += 1; mk("d", "zm3", i, c)
                    if dve is not None:
                        if i >= 2:
                            dve.wait_ge(s_act, marks[("a", "ex", i - 2)])
                        dve.tensor_tensor(
                            out=zd_b[j][:].rearrange("p (k t) -> p k t", k=K),
                            in0=z_kt(jb),
                            in1=zmax[j][:].unsqueeze(1).broadcast_to(
                                (P, K, T)),
                            op=Alu.subtract,
                        ).then_inc(s_dve, 1)
                    c += 1; mk("d", "zd", i, c)
                if 0 <= t < n:
                    jt = t % 2
                    jtb = t % NB
                    if dve is not None:
                        dve.wait_ge(s_act, marks[("a", "ex", t)])
                        dve.tensor_tensor(
                            out=w_b[:], in0=p_t(t), in1=ex_b[jt][:],
                            op=Alu.mult,
                        ).then_inc(s_dve, 1)
                    c += 1; mk("d", "w", t, c)
                    if dve is not None:
                        dve.tensor_tensor(
                            out=wc[:].rearrange("p (c kt) -> p c kt", c=3),
                            in0=inb[jtb][:, TK:IN_W].bitcast(bf16).rearrange(
                                "p (c kt) -> p c kt", c=3),
                            in1=w_b[:].unsqueeze(1).broadcast_to((P, 3, TK)),
                            op=Alu.mult,
                        ).then_inc(s_dve, 1)
                    c += 1; mk("d", "wc", t, c)
                    if dve is not None:
                        dve.tensor_tensor(
                            out=ws4[:], in0=w_b[:, 0:TK // 2],
                            in1=w_b[:, TK // 2:TK], op=Alu.add,
                        ).then_inc(s_dve, 1)
                        dve.tensor_tensor(
                            out=ws2[:], in0=ws4[:, 0:TK // 4],
                            in1=ws4[:, TK // 4:TK // 2], op=Alu.add,
                        ).then_inc(s_dve, 1)
                        dve.tensor_tensor(
                            out=wsum[:], in0=ws2[:, 0:T],
                            in1=ws2[:, T:2 * T], op=Alu.add,
                        ).then_inc(s_dve, 1)
                    c += 3; mk("d", "wsum", t, c)
                    if dve is not None:
                        dve.wait_ge(s_act, marks[("a", "delta", t)])
                        dve.tensor_tensor(
                            out=denom[jt][:], in0=wsum[:], in1=delta[jt][:],
                            op=Alu.add,
                        ).then_inc(s_dve, 1)
                    c += 1; mk("d", "denom", t, c)
                    if dve is not None:
                        dve.wait_ge(s_act, marks[("a", "lnq", t)])
                        dve.tensor_tensor(
                            out=q4[:], in0=q_b[jt][:, 0:TK // 2],
                            in1=q_b[jt][:, TK // 2:TK], op=Alu.add,
                        ).then_inc(s_dve, 1)
                        dve.tensor_tensor(
                            out=q2[:], in0=q4[:, 0:TK // 4],
                            in1=q4[:, TK // 4:TK // 2], op=Alu.add,
                        ).then_inc(s_dve, 1)
                        if t >= 2:
                            dve.wait_ge(s_act, marks[("a", "pq", t - 2)])
                        dve.tensor_tensor(
                            out=qsum[jt][:], in0=q2[:, 0:T],
                            in1=q2[:, T:2 * T], op=Alu.add,
                        ).then_inc(s_dve, 1)
                    c += 3; mk("d", "qsum", t, c)
                    if dve is not None:
                        wcv = wc[:].rearrange("p (c k t) -> p c k t", c=3, k=K)
                        dve.tensor_tensor(
                            out=cs4[:].rearrange("p (c k t) -> p c k t",
                                                 c=3, k=K // 2),
                            in0=wcv[:, :, 0:K // 2, :],
                            in1=wcv[:, :, K // 2:K, :], op=Alu.add,
                        ).then_inc(s_dve, 1)
                        cs4v = cs4[:].rearrange("p (c k t) -> p c k t",
                                                c=3, k=K // 2)
                        dve.tensor_tensor(
                            out=cs2[:].rearrange("p (c k t) -> p c k t",
                                                 c=3, k=K // 4),
                            in0=cs4v[:, :, 0:K // 4, :],
                            in1=cs4v[:, :, K // 4:K // 2, :], op=Alu.add,
                        ).then_inc(s_dve, 1)
                        cs2v = cs2[:].rearrange("p (c k t) -> p c k t",
                                                c=3, k=K // 4)
                        dve.tensor_tensor(
                            out=csum[:].rearrange("p (c t) -> p c t", c=3),
                            in0=cs2v[:, :, 0, :],
                            in1=cs2v[:, :, 1, :], op=Alu.add,
                        ).then_inc(s_dve, 1)
                    c += 3; mk("d", "csum", t, c)
                    if dve is not None:
                        dve.tensor_tensor(
                            out=t3b[jt][:].rearrange("p (c t) -> p c t", c=3),
                            in0=csum[:].rearrange("p (c t) -> p c t", c=3),
                            in1=delta[jt][:].unsqueeze(1).broadcast_to(
                                (P, 3, T)),
                            op=Alu.add,
                        ).then_inc(s_dve, 1)
                    c += 1; mk("d", "t3", t, c)
                if 0 <= u:
                    ju = u % 2
                    if dve is not None:
                        dve.wait_ge(s_act, marks[("a", "rcp", u)])
                        if u >= 2:
                            dve.wait_ge(s_out[ju], 16 * (u // 2))
                        dve.tensor_tensor(
                            out=ot_rgb(ju),
                            in0=t3b[ju][:].rearrange("p (c t) -> p c t", c=3),
                            in1=rcp[ju][:].unsqueeze(1).broadcast_to(
                                (P, 3, T)),
                            op=Alu.mult,
                        ).then_inc(s_dve, 1)
                    c += 1; mk("d", "rgb", u, c)

        # pass 1: record marks
        sched_sp(None)
        sched_act(None)
        sched_dve(None)

        blk = ctx.enter_context(nc.Block())

        @blk.sync
        def _(sp):
            sched_sp(sp)

        @blk.scalar
        def _(act):
            sched_act(act)

        @blk.vector
        def _(dve):
            sched_dve(dve)

    return nc


_CACHE = {}


def _get_program():
    if "nc" not in _CACHE:
        _CACHE["nc"] = build_program()
    return _CACHE["nc"]


def _pack_core(zb, ds, pf, pc, bf16_t):
    """Per-core input: [P, NT*IN_W] u16 blob (z|col) and [P, NT*TK] d."""
    mask = pf >= 0
    z_inv = (ZFAR - zb) * (np.float32(1.0) / (ZFAR - ZNEAR))
    z_inv = np.where(mask, z_inv, np.float32(0.0))
    z16 = np.clip(np.rint(z_inv * np.float32(65535.0)), 0, 65535).astype(
        np.uint16
    )
    d_eff = np.where(mask, ds, np.float32(1.0)).astype(bf16_t).view(np.uint16)

    # pixel p-major: (H*W, K[,3]) -> [P, NT, ...] k-major tiles
    z16 = (
        z16.reshape(P, NT, T, K).transpose(0, 1, 3, 2).reshape(P, NT, TK)
    )
    d16 = (
        d_eff.reshape(P, NT, T, K).transpose(0, 1, 3, 2).reshape(P, NT * TK)
    )
    c16 = (
        pc.astype(bf16_t)
        .view(np.uint16)
        .reshape(P, NT, T, K, 3)
        .transpose(0, 1, 4, 3, 2)
        .reshape(P, NT, TK * 3)
    )
    blob = np.ascontiguousarray(
        np.concatenate([z16, c16], axis=2)
    ).reshape(P, NT * IN_W)
    return blob, np.ascontiguousarray(d16)


def _run(pixel_colors, zbuf, dists, pix_to_face, trace=False):
    import ml_dtypes
    from concourse.bass_utils import run_bass_kernel_spmd

    bf16_t = ml_dtypes.bfloat16

    N, H, W, Kk = zbuf.shape
    assert (N, H, W, Kk) == (N_CORES, 512, 512, K), (N, H, W, Kk)

    nc = _get_program()

    pc = np.asarray(pixel_colors, dtype=np.float32)
    zb = np.asarray(zbuf, dtype=np.float32)
    ds = np.asarray(dists, dtype=np.float32)
    pf = np.asarray(pix_to_face)

    in_maps = []
    for i in range(N_CORES):
        blob, din = _pack_core(
            zb[i].reshape(-1, K),
            ds[i].reshape(-1, K),
            pf[i].reshape(-1, K),
            pc[i].reshape(-1, K, 3),
            bf16_t,
        )
        in_maps.append({"inb": blob, "din": din})

    res = run_bass_kernel_spmd(
        nc, in_maps, core_ids=list(range(N_CORES)), trace=trace
    )
    outs = []
    for i in range(N_CORES):
        o = res.results[i]["out"]  # [P, NT*OUT_W] u16
        o = (
            np.ascontiguousarray(o)
            .view(bf16_t)
            .reshape(P, NT, 4, T)
            .transpose(0, 1, 3, 2)
            .astype(np.float32)
            .reshape(H, W, 4)
        )
        outs.append(o)
    return np.stack(outs, axis=0), res


def kernel(pixel_colors, zbuf, dists, pix_to_face):
    out, _ = _run(pixel_colors, zbuf, dists, pix_to_face, trace=False)
    return out


# revision 46
# speedup vs baseline: 1.0327x; 1.0327x over previous
"""Trainium2 Bass kernel for softmax RGB blend (pytorch3d NoLightShader).

Full inputs (N=8, H=512, W=512, K=8) are sharded batch-wise across 8
NeuronCores (one image per core); the blend is per-pixel, no cross-core
communication.

Host-side input encoding (per core):
    mask folded into the data (pix_to_face never shipped):
        d_eff = where(mask, dists, 1.0)        -> sigmoid(-d/SIGMA) = 0
        z_inv = (ZFAR - zbuf)/(ZFAR - ZNEAR) * mask
    z shipped as uint16 fixed point (z16 = round(65535 * z_inv)): u16 order
    matches float order, so the K-max runs in u16, and ACT's free affine
    (scale/bias) turns u16 straight into exp arguments.
    dists/colors shipped as bf16. Per-tile layout is k-major [K, T] (colors
    [3, K, T]) so every K-reduction is a contiguous pairwise fold tree at
    DVE 2x bf16 mode (tensor_reduce is stuck at 1x). dists ship as one
    up-front stream so ALL sigmoids run in a prepass -- the sigmoid and
    ln/exp ACT table sets otherwise swap twice per tile (~2.7us a load).
    Output is planar bf16 [4, T] per tile (r|g|b|a), host transposes.

Math per pixel:  p_k = sigmoid(-d_k/SIGMA); q_k = 1-p_k
    alpha = 1 - prod_k q_k     (DVE computes mq=p-1; GPSIMD mult fold tree;
                                8 negations cancel)
    zmax  = max_k z_k          (DVE u16 max fold tree)
    w_k   = p_k * exp((z_k - zmax)/GAMMA)  (zd=zmax-z fp16 on GPSIMD, exp ACT)
    delta = exp((EPS - zmax)/GAMMA)
    denom = sum_k w_k + delta              (DVE bf16 add fold tree)
    rgb   = (sum_k w_k c_k + delta)/denom  (bg=1; wc + fold tree on DVE)
    out   = [rgb, alpha]

Engines: SP HWDGE DMAs (d-stream + 1 in + 1 out per tile) | ACT: sigmoid
prepass, exp(zd), delta, ln(denom), rcp=exp(-ln), alpha | DVE: zmax folds,
mq, w, wc, wsum folds, denom, csum folds, t3, rgb | GPSIMD: zd, prod-q
folds. Raw bass, two-pass mark/wait scheduling, double-buffered tiles.
"""

import sys
from contextlib import ExitStack

import numpy as np

if "/opt/trn_rl_repo" not in sys.path:
    sys.path.insert(0, "/opt/trn_rl_repo")

SIGMA = 1e-4
GAMMA = 1e-4
ZNEAR = 1.0
ZFAR = 100.0
EPS = 1e-10

P = 128
K = 8
N_CORES = 8
ROWS = 2048          # H*W / P
T = 256              # pixels per partition per tile
NT = ROWS // T       # 8 tiles
TK = T * K           # 2048
IN_W = TK + TK * 3        # u16 words per tile: z | col
OUT_W = T * 4             # bf16 words per tile (planar r|g|b|a)

S16G = (1.0 / 65535.0) / GAMMA   # u16 step -> 1/GAMMA units


def build_program():
    import concourse.bass as bass
    from concourse import mybir

    dt = mybir.dt
    f32 = dt.float32
    bf16 = dt.bfloat16
    fp16 = dt.float16
    u16 = dt.uint16
    Alu = mybir.AluOpType
    Act = mybir.ActivationFunctionType

    n = NT

    nc = bass.Bass()

    in_d = nc.dram_tensor("inb", [P, n * IN_W], u16, kind="ExternalInput")
    d_d = nc.dram_tensor("din", [P, n * TK], u16, kind="ExternalInput")
    out_d = nc.dram_tensor("out", [P, n * OUT_W], u16, kind="ExternalOutput")

    # const AP for the delta bias (EPS/GAMMA); framework pre-registers 0.0/1.0.
    # Written by the first DVE op; every ACT reader (delta) transitively waits
    # on later DVE marks, so no barrier is needed.
    cbias = nc.alloc_sbuf_tensor("c_epsg", [P, 1], f32)
    nc.const_aps.aps[(f32, EPS / GAMMA)] = cbias.ap()

    with ExitStack() as ctx:
        def sb(name, w, dty=bf16):
            return ctx.enter_context(nc.sbuf_tensor(name, [P, w], dty))

        NB = 3  # input tile buffers
        inb = [sb(f"inb{j}", IN_W, u16) for j in range(NB)]
        d_sb = sb("dall", n * TK, u16)   # d bf16; sigmoid overwrites in place
        ot = [sb(f"ot{j}", OUT_W, u16) for j in range(2)]

        q_b = [sb(f"q{j}", TK) for j in range(2)]
        ex_b = [sb(f"ex{j}", TK) for j in range(2)]
        zd_b = [sb(f"zd{j}", TK, fp16) for j in range(2)]
        zmax = [sb(f"zmax{j}", T, u16) for j in range(2)]
        delta = [sb(f"delta{j}", T) for j in range(2)]
        qsum = [sb(f"qsum{j}", T, f32) for j in range(2)]
        pqt = sb("pqt", T, f32)
        rcp = [sb(f"rcp{j}", T) for j in range(2)]
        t3b = [sb(f"t3{j}", T * 3) for j in range(2)]
        denom = [sb(f"denom{j}", T, f32) for j in range(2)]

        zm4 = sb("zm4", TK // 2, u16)
        zm2 = sb("zm2", TK // 4, u16)
        w_b = sb("w", TK)
        ws4 = sb("ws4", TK // 2)
        ws2 = sb("ws2", TK // 4)
        wsum = sb("wsum", T)
        q4 = sb("q4", TK // 2)
        q2 = sb("q2", TK // 4)
        wc = sb("wc", TK * 3)
        cs4 = sb("cs4", TK * 3 // 2)
        cs2 = sb("cs2", TK * 3 // 4)
        csum = sb("csum", T * 3)
        lnden = sb("lnden", T, f32)

        s_in = [
            ctx.enter_context(nc.semaphore("s_in0")),
            ctx.enter_context(nc.semaphore("s_in1")),
            ctx.enter_context(nc.semaphore("s_in2")),
        ]
        s_out = [
            ctx.enter_context(nc.semaphore("s_out0")),
            ctx.enter_context(nc.semaphore("s_out1")),
        ]
        s_ind = [
            ctx.enter_context(nc.semaphore(f"s_ind{j}")) for j in range(4)
        ]
        s_act = ctx.enter_context(nc.semaphore("s_act"))
        s_dve = ctx.enter_context(nc.semaphore("s_dve"))
        s_gp = ctx.enter_context(nc.semaphore("s_gp"))

        marks = {}

        def mk(engkey, name, t, ctr):
            marks[(engkey, name, t)] = ctr

        # ---- SBUF views -------------------------------------------------
        def z_kt(j):      # [P, K, T] u16
            return inb[j][:, 0:TK].rearrange("p (k t) -> p k t", k=K)

        def col_ckt(j):   # [P, 3, K, T] bf16
            return inb[j][:, TK:IN_W].bitcast(bf16).rearrange(
                "p (c k t) -> p c k t", c=3, k=K
            )

        def d_bf(i):      # [P, TK] bf16, tile i of the d stream
            return d_sb[:, bass.ts(i, TK)].bitcast(bf16)

        def p_t(i):       # [P, TK] bf16, tile i of sigmoid (in-place over d)
            return d_sb[:, bass.ts(i, TK)].bitcast(bf16)

        def ot_rgb(j):    # [P, 3, T] bf16 planar
            return ot[j][:, 0:3 * T].bitcast(bf16).rearrange(
                "p (c t) -> p c t", c=3
            )

        def ot_a(j):      # [P, T] bf16
            return ot[j][:, 3 * T:4 * T].bitcast(bf16)

        # ---- schedules --------------------------------------------------
        def sched_sp(sp):
            if sp is not None:
                # interleave the first input tiles with d quarters; each d
                # quarter gets its own FULL-value sem so prepass chunks can
                # start as soon as their d lands
                q = n * TK // 4
                sp.dma_start(
                    out=inb[0][:], in_=in_d[:, bass.ts(0, IN_W)]
                ).then_inc(s_in[0], 16)
                sp.dma_start(out=d_sb[:, 0:q], in_=d_d[:, 0:q]
                             ).then_inc(s_ind[0], 16)
                sp.dma_start(
                    out=inb[1][:], in_=in_d[:, bass.ts(1, IN_W)]
                ).then_inc(s_in[1], 16)
                sp.dma_start(out=d_sb[:, q:2 * q], in_=d_d[:, q:2 * q]
                             ).then_inc(s_ind[1], 16)
                sp.dma_start(out=d_sb[:, 2 * q:3 * q], in_=d_d[:, 2 * q:3 * q]
                             ).then_inc(s_ind[2], 16)
                sp.dma_start(
                    out=inb[2][:], in_=in_d[:, bass.ts(2, IN_W)]
                ).then_inc(s_in[2], 16)
                sp.dma_start(out=d_sb[:, 3 * q:4 * q], in_=d_d[:, 3 * q:4 * q]
                             ).then_inc(s_ind[3], 16)
            for i in range(NB, n):
                j = i % NB
                if sp is not None:
                    sp.wait_ge(s_dve, marks[("d", "wc", i - NB)])
                    sp.dma_start(
                        out=inb[j][:], in_=in_d[:, bass.ts(i, IN_W)]
                    ).then_inc(s_in[j], 16)
            if sp is not None:
                sp.wait_ge(s_out[0], 16 * ((n + 1) // 2))
                sp.wait_ge(s_out[1], 16 * (n // 2))

        def emit_ex(act, t):
            act.activation(ex_b[t % 2][:], zd_b[t % 2][:], Act.Exp,
                           scale=S16G).then_inc(s_act, 1)

        def emit_delta(act, t):
            act.activation(
                delta[t % 2][:], zmax[t % 2][:], Act.Exp,
                bias=EPS / GAMMA, scale=-S16G,
            ).then_inc(s_act, 1)

        def emit_lnq(act, t):
            act.activation(q_b[t % 2][:], p_t(t), Act.Ln,
                           bias=1.0, scale=-1.0).then_inc(s_act, 1)

        def sched_act(act):
            c = 0
            # sigmoid prepass in 2-tile chunks (one table set); each early
            # tile's exp/ln group is interleaved after its chunk so DVE's
            # w(t) isn't blocked on the whole prepass
            for ch in range(n // 2):
                if ch < 3:
                    # exp/delta for tile ch (zd ready from DVE) plus the
                    # PREVIOUS tile's lnq (its sigmoid chunk is done)
                    t = ch
                    if t >= 1:
                        if act is not None:
                            emit_lnq(act, t - 1)
                        c += 1; mk("a", "lnq", t - 1, c)
                    if act is not None:
                        act.wait_ge(s_dve, marks[("d", "zd", t)])
                        if t >= 2:
                            act.wait_ge(s_dve, marks[("d", "w", t - 2)])
                        emit_ex(act, t)
                    c += 1; mk("a", "ex", t, c)
                    if act is not None:
                        if t >= 2:
                            act.wait_ge(s_dve, marks[("d", "t3", t - 2)])
                        emit_delta(act, t)
                    c += 1; mk("a", "delta", t, c)
                if act is not None:
                    act.wait_ge(s_ind[ch], 16)
                    act.activation(
                        d_sb[:, bass.ts(ch, 2 * TK)].bitcast(bf16),
                        d_sb[:, bass.ts(ch, 2 * TK)].bitcast(bf16),
                        Act.Sigmoid, scale=-1.0 / SIGMA,
                    ).then_inc(s_act, 1)
                c += 1
                mk("a", "p", 2 * ch, c)
                mk("a", "p", 2 * ch + 1, c)
            if act is not None:
                emit_lnq(act, 2)
            c += 1; mk("a", "lnq", 2, c)
            for i in range(n + 2):
                t = i - 1
                u = i - 2
                if 3 <= t < n:
                    if act is not None:
                        act.wait_ge(s_dve, marks[("d", "zd", t)])
                        if t >= 2:
                            act.wait_ge(s_dve, marks[("d", "w", t - 2)])
                        emit_ex(act, t)
                    c += 1; mk("a", "ex", t, c)
                    if act is not None:
                        if t >= 2:
                            act.wait_ge(s_dve, marks[("d", "t3", t - 2)])
                        emit_delta(act, t)
                    c += 1; mk("a", "delta", t, c)
                    if act is not None:
                        emit_lnq(act, t)
                    c += 1; mk("a", "lnq", t, c)
                if u >= 0:
                    if act is not None:
                        act.wait_ge(s_dve, marks[("d", "denom", u)])
                        act.activation(lnden[:], denom[u % 2][:], Act.Ln
                                       ).then_inc(s_act, 1)
                    c += 1; mk("a", "lnd", u, c)
                    if act is not None:
                        if u >= 2:
                            act.wait_ge(s_dve, marks[("d", "rgb", u - 2)])
                        act.activation(rcp[u % 2][:], lnden[:], Act.Exp,
                                       scale=-1.0).then_inc(s_act, 1)
                    c += 1; mk("a", "rcp", u, c)
                    if act is not None:
                        act.wait_ge(s_dve, marks[("d", "qsum", u)])
                        act.activation(pqt[:], qsum[u % 2][:], Act.Exp
                                       ).then_inc(s_act, 1)
                    c += 1; mk("a", "pq", u, c)
                    if act is not None:
                        if u >= 2:
                            act.wait_ge(s_out[u % 2], 16 * (u // 2))
                        act.activation(ot_a(u % 2), pqt[:], Act.Copy,
                                       bias=1.0, scale=-1.0).then_inc(s_act, 1)
                    c += 1; mk("a", "alpha", u, c)
                    if act is not None:
                        act.wait_ge(s_dve, marks[("d", "rgb", u)])
                        act.dma_start(
                            out=out_d[:, bass.ts(u, OUT_W)], in_=ot[u % 2][:]
                        ).then_inc(s_out[u % 2], 16)

        def sched_dve(dve):
            c = 0
            if dve is not None:
                dve.memset(cbias.ap(), EPS / GAMMA)
            for i in range(n + 2):
                t = i - 1
                u = i - 2
                if i < n:
                    j = i % 2
                    jb = i % NB
                    if dve is not None:
                        dve.wait_ge(s_in[jb], 16 * (i // NB + 1))
                        if i >= 2:
                            dve.wait_ge(s_act, marks[("a", "delta", i - 2)])
                        zv = inb[jb][:, 0:TK]
                        dve.tensor_tensor(
                            out=zm4[:], in0=zv[:, 0:TK // 2],
                            in1=zv[:, TK // 2:TK], op=Alu.max,
                        ).then_inc(s_dve, 1)
                    c += 1; mk("d", "zm1", i, c)
                    if dve is not None:
                        dve.tensor_tensor(
                            out=zm2[:], in0=zm4[:, 0:TK // 4],
                            in1=zm4[:, TK // 4:TK // 2], op=Alu.max,
                        ).then_inc(s_dve, 1)
                    c += 1; mk("d", "zm2", i, c)
                    if dve is not None:
                        dve.tensor_tensor(
                            out=zmax[j][:], in0=zm2[:, 0:T],
                            in1=zm2[:, T:2 * T], op=Alu.max,
                        ).then_inc(s_dve, 1)
                    c += 1; mk("d", "zm3", i, c)
                    if dve is not None:
                        if i >= 2:
                            dve.wait_ge(s_act, marks[("a", "ex", i - 2)])
                        dve.tensor_tensor(
                            out=zd_b[j][:].rearrange("p (k t) -> p k t", k=K),
                            in0=z_kt(jb),
                            in1=zmax[j][:].unsqueeze(1).broadcast_to(
                                (P, K, T)),
                            op=Alu.subtract,
                        ).then_inc(s_dve, 1)
                    c += 1; mk("d", "zd", i, c)
                if 0 <= t < n:
                    jt = t % 2
                    jtb = t % NB
                    if dve is not None:
                        dve.wait_ge(s_act, marks[("a", "ex", t)])
                        dve.tensor_tensor(
                            out=w_b[:], in0=p_t(t), in1=ex_b[jt][:],
                            op=Alu.mult,
                        ).then_inc(s_dve, 1)
                    c += 1; mk("d", "w", t, c)
                    if dve is not None:
                        dve.tensor_tensor(
                            out=wc[:].rearrange("p (c kt) -> p c kt", c=3),
                            in0=inb[jtb][:, TK:IN_W].bitcast(bf16).rearrange(
                                "p (c kt) -> p c kt", c=3),
                            in1=w_b[:].unsqueeze(1).broadcast_to((P, 3, TK)),
                            op=Alu.mult,
                        ).then_inc(s_dve, 1)
                    c += 1; mk("d", "wc", t, c)
                    if dve is not None:
                        dve.tensor_tensor(
                            out=ws4[:], in0=w_b[:, 0:TK // 2],
                            in1=w_b[:, TK // 2:TK], op=Alu.add,
                        ).then_inc(s_dve, 1)
                        dve.tensor_tensor(
                            out=ws2[:], in0=ws4[:, 0:TK // 4],
                            in1=ws4[:, TK // 4:TK // 2], op=Alu.add,
                        ).then_inc(s_dve, 1)
                        dve.tensor_tensor(
                            out=wsum[:], in0=ws2[:, 0:T],
                            in1=ws2[:, T:2 * T], op=Alu.add,
                        ).then_inc(s_dve, 1)
                    c += 3; mk("d", "wsum", t, c)
                    if dve is not None:
                        dve.wait_ge(s_act, marks[("a", "delta", t)])
                        dve.tensor_tensor(
                            out=denom[jt][:], in0=wsum[:], in1=delta[jt][:],
                            op=Alu.add,
                        ).then_inc(s_dve, 1)
                    c += 1; mk("d", "denom", t, c)
                    if dve is not None:
                        wcv = wc[:].rearrange("p (c k t) -> p c k t", c=3, k=K)
                        dve.tensor_tensor(
                            out=cs4[:].rearrange("p (c k t) -> p c k t",
                                                 c=3, k=K // 2),
                            in0=wcv[:, :, 0:K // 2, :],
                            in1=wcv[:, :, K // 2:K, :], op=Alu.add,
                        ).then_inc(s_dve, 1)
                        cs4v = cs4[:].rearrange("p (c k t) -> p c k t",
                                                c=3, k=K // 2)
                        dve.tensor_tensor(
                            out=cs2[:].rearrange("p (c k t) -> p c k t",
                                                 c=3, k=K // 4),
                            in0=cs4v[:, :, 0:K // 4, :],
                            in1=cs4v[:, :, K // 4:K // 2, :], op=Alu.add,
                        ).then_inc(s_dve, 1)
                        cs2v = cs2[:].rearrange("p (c k t) -> p c k t",
                                                c=3, k=K // 4)
                        dve.tensor_tensor(
                            out=csum[:].rearrange("p (c t) -> p c t", c=3),
                            in0=cs2v[:, :, 0, :],
                            in1=cs2v[:, :, 1, :], op=Alu.add,
                        ).then_inc(s_dve, 1)
                    c += 3; mk("d", "csum", t, c)
                    if dve is not None:
                        dve.tensor_tensor(
                            out=t3b[jt][:].rearrange("p (c t) -> p c t", c=3),
                            in0=csum[:].rearrange("p (c t) -> p c t", c=3),
                            in1=delta[jt][:].unsqueeze(1).broadcast_to(
                                (P, 3, T)),
                            op=Alu.add,
                        ).then_inc(s_dve, 1)
                    c += 1; mk("d", "t3", t, c)
                    if dve is not None:
                        dve.wait_ge(s_act, marks[("a", "lnq", t)])
                        dve.tensor_tensor(
                            out=q4[:], in0=q_b[jt][:, 0:TK // 2],
                            in1=q_b[jt][:, TK // 2:TK], op=Alu.add,
                        ).then_inc(s_dve, 1)
                        dve.tensor_tensor(
                            out=q2[:], in0=q4[:, 0:TK // 4],
                            in1=q4[:, TK // 4:TK // 2], op=Alu.add,
                        ).then_inc(s_dve, 1)
                        if t >= 2:
                            dve.wait_ge(s_act, marks[("a", "pq", t - 2)])
                        dve.tensor_tensor(
                            out=qsum[jt][:], in0=q2[:, 0:T],
                            in1=q2[:, T:2 * T], op=Alu.add,
                        ).then_inc(s_dve, 1)
                    c += 3; mk("d", "qsum", t, c)
                if 0 <= u:
                    ju = u % 2
                    if dve is not None:
                        dve.wait_ge(s_act, marks[("a", "rcp", u)])
                        if u >= 2:
                            dve.wait_ge(s_out[ju], 16 * (u // 2))
                        dve.tensor_tensor(
                            out=ot_rgb(ju),
                            in0=t3b[ju][:].rearrange("p (c t) -> p c t", c=3),
                            in1=rcp[ju][:].unsqueeze(1).broadcast_to(
                                (P, 3, T)),
                            op=Alu.mult,
                        ).then_inc(s_dve, 1)
                    c += 1; mk("d", "rgb", u, c)

        # pass 1: record marks
        sched_sp(None)
        sched_act(None)
        sched_dve(None)

        blk = ctx.enter_context(nc.Block())

        @blk.sync
        def _(sp):
            sched_sp(sp)

        @blk.scalar
        def _(act):
            sched_act(act)

        @blk.vector
        def _(dve):
            sched_dve(dve)

    return nc


_CACHE = {}


def _get_program():
    if "nc" not in _CACHE:
        _CACHE["nc"] = build_program()
    return _CACHE["nc"]


def _pack_core(zb, ds, pf, pc, bf16_t):
    """Per-core input: [P, NT*IN_W] u16 blob (z|col) and [P, NT*TK] d."""
    mask = pf >= 0
    z_inv = (ZFAR - zb) * (np.float32(1.0) / (ZFAR - ZNEAR))
    z_inv = np.where(mask, z_inv, np.float32(0.0))
    z16 = np.clip(np.rint(z_inv * np.float32(65535.0)), 0, 65535).astype(
        np.uint16
    )
    d_eff = np.where(mask, ds, np.float32(1.0)).astype(bf16_t).view(np.uint16)

    # pixel p-major: (H*W, K[,3]) -> [P, NT, ...] k-major tiles
    z16 = (
        z16.reshape(P, NT, T, K).transpose(0, 1, 3, 2).reshape(P, NT, TK)
    )
    d16 = (
        d_eff.reshape(P, NT, T, K).transpose(0, 1, 3, 2).reshape(P, NT * TK)
    )
    c16 = (
        pc.astype(bf16_t)
        .view(np.uint16)
        .reshape(P, NT, T, K, 3)
        .transpose(0, 1, 4, 3, 2)
        .reshape(P, NT, TK * 3)
    )
    blob = np.ascontiguousarray(
        np.concatenate([z16, c16], axis=2)
    ).reshape(P, NT * IN_W)
    return blob, np.ascontiguousarray(d16)


def _run(pixel_colors, zbuf, dists, pix_to_face, trace=False):
    import ml_dtypes
    from concourse.bass_utils import run_bass_kernel_spmd

    bf16_t = ml_dtypes.bfloat16

    N, H, W, Kk = zbuf.shape
    assert (N, H, W, Kk) == (N_CORES, 512, 512, K), (N, H, W, Kk)

    nc = _get_program()

    pc = np.asarray(pixel_colors, dtype=np.float32)
    zb = np.asarray(zbuf, dtype=np.float32)
    ds = np.asarray(dists, dtype=np.float32)
    pf = np.asarray(pix_to_face)

    in_maps = []
    for i in range(N_CORES):
        blob, din = _pack_core(
            zb[i].reshape(-1, K),
            ds[i].reshape(-1, K),
            pf[i].reshape(-1, K),
            pc[i].reshape(-1, K, 3),
            bf16_t,
        )
        in_maps.append({"inb": blob, "din": din})

    res = run_bass_kernel_spmd(
        nc, in_maps, core_ids=list(range(N_CORES)), trace=trace
    )
    outs = []
    for i in range(N_CORES):
        o = res.results[i]["out"]  # [P, NT*OUT_W] u16
        o = (
            np.ascontiguousarray(o)
            .view(bf16_t)
            .reshape(P, NT, 4, T)
            .transpose(0, 1, 3, 2)
            .astype(np.float32)
            .reshape(H, W, 4)
        )
        outs.append(o)
    return np.stack(outs, axis=0), res


def kernel(pixel_colors, zbuf, dists, pix_to_face):
    out, _ = _run(pixel_colors, zbuf, dists, pix_to_face, trace=False)
    return out


# revision 55
# speedup vs baseline: 1.1015x; 1.0665x over previous
"""Trainium2 Bass kernel for softmax RGB blend (pytorch3d NoLightShader).

Full inputs (N=8, H=512, W=512, K=8) are sharded batch-wise across 8
NeuronCores (one image per core); the blend is per-pixel, no cross-core
communication.

Host-side input encoding (per core):
    mask folded into the data (pix_to_face never shipped):
        d_eff = where(mask, dists, 1.0)        -> sigmoid(-d/SIGMA) = 0
        z_inv = (ZFAR - zbuf)/(ZFAR - ZNEAR) * mask
    z shipped as uint16 fixed point (z16 = round(65535 * z_inv)): u16 order
    matches float order, so the K-max runs in u16, and ACT's free affine
    (scale/bias) turns u16 straight into exp arguments.
    dists/colors shipped as bf16. Per-tile layout is k-major [K, T] (colors
    [3, K, T]) so every K-reduction is a contiguous pairwise fold tree at
    DVE 2x bf16 mode (tensor_reduce is stuck at 1x). dists ship as one
    up-front stream so ALL sigmoids run in a prepass -- the sigmoid and
    ln/exp ACT table sets otherwise swap twice per tile (~2.7us a load).
    Output is planar bf16 [4, T] per tile (r|g|b|a), host transposes.

Math per pixel:  p_k = sigmoid(-d_k/SIGMA); q_k = 1-p_k
    alpha = 1 - prod_k q_k     (DVE computes mq=p-1; GPSIMD mult fold tree;
                                8 negations cancel)
    zmax  = max_k z_k          (DVE u16 max fold tree)
    w_k   = p_k * exp((z_k - zmax)/GAMMA)  (zd=zmax-z fp16 on GPSIMD, exp ACT)
    delta = exp((EPS - zmax)/GAMMA)
    denom = sum_k w_k + delta              (DVE bf16 add fold tree)
    rgb   = (sum_k w_k c_k + delta)/denom  (bg=1; wc + fold tree on DVE)
    out   = [rgb, alpha]

Engines: SP HWDGE DMAs (d-stream + 1 in + 1 out per tile) | ACT: sigmoid
prepass, exp(zd), delta, ln(denom), rcp=exp(-ln), alpha | DVE: zmax folds,
mq, w, wc, wsum folds, denom, csum folds, t3, rgb | GPSIMD: zd, prod-q
folds. Raw bass, two-pass mark/wait scheduling, double-buffered tiles.
"""

import sys
from contextlib import ExitStack

import numpy as np

if "/opt/trn_rl_repo" not in sys.path:
    sys.path.insert(0, "/opt/trn_rl_repo")

SIGMA = 1e-4
GAMMA = 1e-4
ZNEAR = 1.0
ZFAR = 100.0
EPS = 1e-10

P = 128
K = 8
N_CORES = 8
ROWS = 2048          # H*W / P
T = 256              # pixels per partition per tile
NT = ROWS // T       # 8 tiles
TK = T * K           # 2048
IN_W = TK + TK * 3        # u16 words per tile: z | col
OUT_W = T * 4             # bf16 words per tile (planar r|g|b|a)

S16G = (1.0 / 65535.0) / GAMMA   # u16 step -> 1/GAMMA units


def build_program():
    import concourse.bass as bass
    from concourse import mybir

    dt = mybir.dt
    f32 = dt.float32
    bf16 = dt.bfloat16
    fp16 = dt.float16
    u16 = dt.uint16
    Alu = mybir.AluOpType
    Act = mybir.ActivationFunctionType

    n = NT

    nc = bass.Bass()

    in_d = nc.dram_tensor("inb", [P, n * IN_W], u16, kind="ExternalInput")
    d_d = nc.dram_tensor("din", [P, n * TK], u16, kind="ExternalInput")
    out_d = nc.dram_tensor("out", [P, n * OUT_W], u16, kind="ExternalOutput")

    # delta's EPS/GAMMA bias (1e-6, a relative 1e-6 scale on delta) is
    # dropped: it is far below the bf16 noise floor and avoiding it means
    # no custom const AP needs initializing before the ACT prepass.

    with ExitStack() as ctx:
        def sb(name, w, dty=bf16):
            return ctx.enter_context(nc.sbuf_tensor(name, [P, w], dty))

        NB = 3  # input tile buffers
        inb = [sb(f"inb{j}", IN_W, u16) for j in range(NB)]
        d_sb = sb("dall", n * TK, u16)   # d bf16; sigmoid overwrites in place
        ot = [sb(f"ot{j}", OUT_W, u16) for j in range(2)]

        q_b = [sb(f"q{j}", TK) for j in range(2)]
        ex_b = [sb(f"ex{j}", TK) for j in range(2)]
        zd_b = [sb(f"zd{j}", TK, fp16) for j in range(2)]
        zmax = [sb(f"zmax{j}", T, u16) for j in range(2)]
        delta = [sb(f"delta{j}", T) for j in range(2)]
        qsum = [sb(f"qsum{j}", T, f32) for j in range(2)]
        pqt = sb("pqt", T, f32)
        rcp = [sb(f"rcp{j}", T) for j in range(2)]
        t3b = [sb(f"t3{j}", T * 3) for j in range(2)]
        denom = [sb(f"denom{j}", T, f32) for j in range(2)]

        zm4 = sb("zm4", TK // 2, u16)
        zm2 = sb("zm2", TK // 4, u16)
        w_b = sb("w", TK)
        ws4 = sb("ws4", TK // 2)
        ws2 = sb("ws2", TK // 4)
        wsum = sb("wsum", T)
        q4 = sb("q4", TK // 2)
        q2 = sb("q2", TK // 4)
        wc = sb("wc", TK * 3)
        cs4 = sb("cs4", TK * 3 // 2)
        cs2 = sb("cs2", TK * 3 // 4)
        csum = sb("csum", T * 3)
        lnden = sb("lnden", T, f32)

        s_in = [
            ctx.enter_context(nc.semaphore("s_in0")),
            ctx.enter_context(nc.semaphore("s_in1")),
            ctx.enter_context(nc.semaphore("s_in2")),
        ]
        s_out = [
            ctx.enter_context(nc.semaphore("s_out0")),
            ctx.enter_context(nc.semaphore("s_out1")),
        ]
        s_ind = [
            ctx.enter_context(nc.semaphore(f"s_ind{j}")) for j in range(4)
        ]
        s_inz = [
            ctx.enter_context(nc.semaphore(f"s_inz{j}")) for j in range(2)
        ]
        s_act = ctx.enter_context(nc.semaphore("s_act"))
        s_dve = ctx.enter_context(nc.semaphore("s_dve"))
        s_gp = ctx.enter_context(nc.semaphore("s_gp"))

        marks = {}

        def mk(engkey, name, t, ctr):
            marks[(engkey, name, t)] = ctr

        # ---- SBUF views -------------------------------------------------
        def z_kt(j):      # [P, K, T] u16
            return inb[j][:, 0:TK].rearrange("p (k t) -> p k t", k=K)

        def col_ckt(j):   # [P, 3, K, T] bf16
            return inb[j][:, TK:IN_W].bitcast(bf16).rearrange(
                "p (c k t) -> p c k t", c=3, k=K
            )

        def d_bf(i):      # [P, TK] bf16, tile i of the d stream
            return d_sb[:, bass.ts(i, TK)].bitcast(bf16)

        def p_t(i):       # [P, TK] bf16, tile i of sigmoid (in-place over d)
            return d_sb[:, bass.ts(i, TK)].bitcast(bf16)

        def ot_rgb(j):    # [P, 3, T] bf16 planar
            return ot[j][:, 0:3 * T].bitcast(bf16).rearrange(
                "p (c t) -> p c t", c=3
            )

        def ot_a(j):      # [P, T] bf16
            return ot[j][:, 3 * T:4 * T].bitcast(bf16)

        # ---- schedules --------------------------------------------------
        def sched_sp(sp):
            if sp is not None:
                # interleave the first input tiles with d quarters; each d
                # quarter gets its own FULL-value sem so prepass chunks can
                # start as soon as their d lands
                q = n * TK // 4
                # tiles 0/1 split z|col so zm folds start ~6x sooner; the
                # z and col parts get separate sems (full-value waits)
                sp.dma_start(
                    out=inb[0][:, 0:TK], in_=in_d[:, 0:TK]
                ).then_inc(s_inz[0], 16)
                sp.dma_start(out=d_sb[:, 0:q], in_=d_d[:, 0:q]
                             ).then_inc(s_ind[0], 16)
                sp.dma_start(
                    out=inb[1][:, 0:TK], in_=in_d[:, IN_W:IN_W + TK]
                ).then_inc(s_inz[1], 16)
                sp.dma_start(
                    out=inb[0][:, TK:IN_W], in_=in_d[:, TK:IN_W]
                ).then_inc(s_in[0], 16)
                sp.dma_start(out=d_sb[:, q:2 * q], in_=d_d[:, q:2 * q]
                             ).then_inc(s_ind[1], 16)
                sp.dma_start(
                    out=inb[1][:, TK:IN_W],
                    in_=in_d[:, IN_W + TK:2 * IN_W],
                ).then_inc(s_in[1], 16)
                sp.dma_start(out=d_sb[:, 2 * q:3 * q], in_=d_d[:, 2 * q:3 * q]
                             ).then_inc(s_ind[2], 16)
                sp.dma_start(
                    out=inb[2][:], in_=in_d[:, bass.ts(2, IN_W)]
                ).then_inc(s_in[2], 16)
                sp.dma_start(out=d_sb[:, 3 * q:4 * q], in_=d_d[:, 3 * q:4 * q]
                             ).then_inc(s_ind[3], 16)
            for i in range(NB, n):
                j = i % NB
                if sp is not None:
                    sp.wait_ge(s_dve, marks[("d", "wc", i - NB)])
                    sp.dma_start(
                        out=inb[j][:], in_=in_d[:, bass.ts(i, IN_W)]
                    ).then_inc(s_in[j], 16)
            if sp is not None:
                sp.wait_ge(s_out[0], 16 * ((n + 1) // 2))
                sp.wait_ge(s_out[1], 16 * (n // 2))

        def emit_ex(act, t):
            act.activation(ex_b[t % 2][:], zd_b[t % 2][:], Act.Exp,
                           scale=S16G).then_inc(s_act, 1)

        def emit_delta(act, t):
            act.activation(
                delta[t % 2][:], zmax[t % 2][:], Act.Exp, scale=-S16G,
            ).then_inc(s_act, 1)

        def emit_lnq(act, t):
            act.activation(q_b[t % 2][:], p_t(t), Act.Ln,
                           bias=1.0, scale=-1.0).then_inc(s_act, 1)

        def sched_act(act):
            c = 0
            # sigmoid prepass in 2-tile chunks (one table set); each early
            # tile's exp/ln group is interleaved after its chunk so DVE's
            # w(t) isn't blocked on the whole prepass
            for ch in range(n // 2):
                if ch < 3:
                    # exp/delta for tile ch (zd ready from DVE) plus the
                    # PREVIOUS tile's lnq (its sigmoid chunk is done)
                    t = ch
                    if t >= 1:
                        if act is not None:
                            emit_lnq(act, t - 1)
                        c += 1; mk("a", "lnq", t - 1, c)
                    if act is not None:
                        act.wait_ge(s_dve, marks[("d", "zd", t)])
                        if t >= 2:
                            act.wait_ge(s_dve, marks[("d", "w", t - 2)])
                        emit_ex(act, t)
                    c += 1; mk("a", "ex", t, c)
                    if act is not None:
                        if t >= 2:
                            act.wait_ge(s_dve, marks[("d", "t3", t - 2)])
                        emit_delta(act, t)
                    c += 1; mk("a", "delta", t, c)
                if ch == 3:
                    # tile 0's normalize tail, before the last sigmoid chunk
                    # so DVE's rgb(0) isn't blocked on the whole prepass
                    if act is not None:
                        act.wait_ge(s_dve, marks[("d", "denom", 0)])
                        act.activation(lnden[:], denom[0][:], Act.Ln
                                       ).then_inc(s_act, 1)
                    c += 1; mk("a", "lnd", 0, c)
                    if act is not None:
                        act.activation(rcp[0][:], lnden[:], Act.Exp,
                                       scale=-1.0).then_inc(s_act, 1)
                    c += 1; mk("a", "rcp", 0, c)
                    if act is not None:
                        act.wait_ge(s_dve, marks[("d", "qsum", 0)])
                        act.activation(pqt[:], qsum[0][:], Act.Exp
                                       ).then_inc(s_act, 1)
                    c += 1; mk("a", "pq", 0, c)
                    if act is not None:
                        act.activation(ot_a(0), pqt[:], Act.Copy,
                                       bias=1.0, scale=-1.0).then_inc(s_act, 1)
                    c += 1; mk("a", "alpha", 0, c)
                    if act is not None:
                        act.wait_ge(s_dve, marks[("d", "rgb", 0)])
                        act.dma_start(
                            out=out_d[:, bass.ts(0, OUT_W)], in_=ot[0][:]
                        ).then_inc(s_out[0], 16)
                if act is not None:
                    act.wait_ge(s_ind[ch], 16)
                    act.activation(
                        d_sb[:, bass.ts(ch, 2 * TK)].bitcast(bf16),
                        d_sb[:, bass.ts(ch, 2 * TK)].bitcast(bf16),
                        Act.Sigmoid, scale=-1.0 / SIGMA,
                    ).then_inc(s_act, 1)
                c += 1
                mk("a", "p", 2 * ch, c)
                mk("a", "p", 2 * ch + 1, c)
            if act is not None:
                emit_lnq(act, 2)
            c += 1; mk("a", "lnq", 2, c)
            for i in range(n + 2):
                t = i - 1
                u = i - 2
                if 3 <= t < n:
                    if act is not None:
                        act.wait_ge(s_dve, marks[("d", "zd", t)])
                        if t >= 2:
                            act.wait_ge(s_dve, marks[("d", "w", t - 2)])
                        emit_ex(act, t)
                    c += 1; mk("a", "ex", t, c)
                    if act is not None:
                        if t >= 2:
                            act.wait_ge(s_dve, marks[("d", "t3", t - 2)])
                        emit_delta(act, t)
                    c += 1; mk("a", "delta", t, c)
                    if act is not None:
                        emit_lnq(act, t)
                    c += 1; mk("a", "lnq", t, c)
                if u >= 1:
                    if act is not None:
                        act.wait_ge(s_dve, marks[("d", "denom", u)])
                        act.activation(lnden[:], denom[u % 2][:], Act.Ln
                                       ).then_inc(s_act, 1)
                    c += 1; mk("a", "lnd", u, c)
                    if act is not None:
                        if u >= 2:
                            act.wait_ge(s_dve, marks[("d", "rgb", u - 2)])
                        act.activation(rcp[u % 2][:], lnden[:], Act.Exp,
                                       scale=-1.0).then_inc(s_act, 1)
                    c += 1; mk("a", "rcp", u, c)
                    if act is not None:
                        act.wait_ge(s_dve, marks[("d", "qsum", u)])
                        act.activation(pqt[:], qsum[u % 2][:], Act.Exp
                                       ).then_inc(s_act, 1)
                    c += 1; mk("a", "pq", u, c)
                    if act is not None:
                        if u >= 2:
                            act.wait_ge(s_out[u % 2], 16 * (u // 2))
                        act.activation(ot_a(u % 2), pqt[:], Act.Copy,
                                       bias=1.0, scale=-1.0).then_inc(s_act, 1)
                    c += 1; mk("a", "alpha", u, c)
                    if act is not None:
                        act.wait_ge(s_dve, marks[("d", "rgb", u)])
                        act.dma_start(
                            out=out_d[:, bass.ts(u, OUT_W)], in_=ot[u % 2][:]
                        ).then_inc(s_out[u % 2], 16)

        def sched_dve(dve):
            c = 0
            for i in range(n + 2):
                t = i - 1
                u = i - 2
                if i < n:
                    j = i % 2
                    jb = i % NB
                    if dve is not None:
                        if i < 2:
                            dve.wait_ge(s_inz[i], 16)
                        else:
                            dve.wait_ge(s_in[jb], 16 * (i // NB + 1))
                        if i >= 2:
                            dve.wait_ge(s_act, marks[("a", "delta", i - 2)])
                        zv = inb[jb][:, 0:TK]
                        dve.tensor_tensor(
                            out=zm4[:], in0=zv[:, 0:TK // 2],
                            in1=zv[:, TK // 2:TK], op=Alu.max,
                        ).then_inc(s_dve, 1)
                    c += 1; mk("d", "zm1", i, c)
                    if dve is not None:
                        dve.tensor_tensor(
                            out=zm2[:], in0=zm4[:, 0:TK // 4],
                            in1=zm4[:, TK // 4:TK // 2], op=Alu.max,
                        ).then_inc(s_dve, 1)
                    c += 1; mk("d", "zm2", i, c)
                    if dve is not None:
                        dve.tensor_tensor(
                            out=zmax[j][:], in0=zm2[:, 0:T],
                            in1=zm2[:, T:2 * T], op=Alu.max,
                        ).then_inc(s_dve, 1)
                    c += 1; mk("d", "zm3", i, c)
                    if dve is not None:
                        if i >= 2:
                            dve.wait_ge(s_act, marks[("a", "ex", i - 2)])
                        dve.tensor_tensor(
                            out=zd_b[j][:].rearrange("p (k t) -> p k t", k=K),
                            in0=z_kt(jb),
                            in1=zmax[j][:].unsqueeze(1).broadcast_to(
                                (P, K, T)),
                            op=Alu.subtract,
                        ).then_inc(s_dve, 1)
                    c += 1; mk("d", "zd", i, c)
                if 0 <= t < n:
                    jt = t % 2
                    jtb = t % NB
                    if dve is not None:
                        # p(t) may be marked after ex(t) (prepass interleave)
                        dve.wait_ge(s_act, max(marks[("a", "ex", t)],
                                               marks[("a", "p", t)]))
                        dve.tensor_tensor(
                            out=w_b[:], in0=p_t(t), in1=ex_b[jt][:],
                            op=Alu.mult,
                        ).then_inc(s_dve, 1)
                    c += 1; mk("d", "w", t, c)
                    if dve is not None:
                        if t < 2:
                            dve.wait_ge(s_in[t], 16)
                        dve.tensor_tensor(
                            out=wc[:].rearrange("p (c kt) -> p c kt", c=3),
                            in0=inb[jtb][:, TK:IN_W].bitcast(bf16).rearrange(
                                "p (c kt) -> p c kt", c=3),
                            in1=w_b[:].unsqueeze(1).broadcast_to((P, 3, TK)),
                            op=Alu.mult,
                        ).then_inc(s_dve, 1)
                    c += 1; mk("d", "wc", t, c)
                    if dve is not None:
                        dve.tensor_tensor(
                            out=ws4[:], in0=w_b[:, 0:TK // 2],
                            in1=w_b[:, TK // 2:TK], op=Alu.add,
                        ).then_inc(s_dve, 1)
                        dve.tensor_tensor(
                            out=ws2[:], in0=ws4[:, 0:TK // 4],
                            in1=ws4[:, TK // 4:TK // 2], op=Alu.add,
                        ).then_inc(s_dve, 1)
                        dve.tensor_tensor(
                            out=wsum[:], in0=ws2[:, 0:T],
                            in1=ws2[:, T:2 * T], op=Alu.add,
                        ).then_inc(s_dve, 1)
                    c += 3; mk("d", "wsum", t, c)
                    if dve is not None:
                        dve.wait_ge(s_act, marks[("a", "delta", t)])
                        dve.tensor_tensor(
                            out=denom[jt][:], in0=wsum[:], in1=delta[jt][:],
                            op=Alu.add,
                        ).then_inc(s_dve, 1)
                    c += 1; mk("d", "denom", t, c)
                    if dve is not None:
                        wcv = wc[:].rearrange("p (c k t) -> p c k t", c=3, k=K)
                        dve.tensor_tensor(
                            out=cs4[:].rearrange("p (c k t) -> p c k t",
                                                 c=3, k=K // 2),
                            in0=wcv[:, :, 0:K // 2, :],
                            in1=wcv[:, :, K // 2:K, :], op=Alu.add,
                        ).then_inc(s_dve, 1)
                        cs4v = cs4[:].rearrange("p (c k t) -> p c k t",
                                                c=3, k=K // 2)
                        dve.tensor_tensor(
                            out=cs2[:].rearrange("p (c k t) -> p c k t",
                                                 c=3, k=K // 4),
                            in0=cs4v[:, :, 0:K // 4, :],
                            in1=cs4v[:, :, K // 4:K // 2, :], op=Alu.add,
                        ).then_inc(s_dve, 1)
                        cs2v = cs2[:].rearrange("p (c k t) -> p c k t",
                                                c=3, k=K // 4)
                        dve.tensor_tensor(
                            out=csum[:].rearrange("p (c t) -> p c t", c=3),
                            in0=cs2v[:, :, 0, :],
                            in1=cs2v[:, :, 1, :], op=Alu.add,
                        ).then_inc(s_dve, 1)
                    c += 3; mk("d", "csum", t, c)
                    if dve is not None:
                        dve.tensor_tensor(
                            out=t3b[jt][:].rearrange("p (c t) -> p c t", c=3),
                            in0=csum[:].rearrange("p (c t) -> p c t", c=3),
                            in1=delta[jt][:].unsqueeze(1).broadcast_to(
                                (P, 3, T)),
                            op=Alu.add,
                        ).then_inc(s_dve, 1)
                    c += 1; mk("d", "t3", t, c)
                    if dve is not None:
                        dve.wait_ge(s_act, marks[("a", "lnq", t)])
                        dve.tensor_tensor(
                            out=q4[:], in0=q_b[jt][:, 0:TK // 2],
                            in1=q_b[jt][:, TK // 2:TK], op=Alu.add,
                        ).then_inc(s_dve, 1)
                        dve.tensor_tensor(
                            out=q2[:], in0=q4[:, 0:TK // 4],
                            in1=q4[:, TK // 4:TK // 2], op=Alu.add,
                        ).then_inc(s_dve, 1)
                        if t >= 2:
                            dve.wait_ge(s_act, marks[("a", "pq", t - 2)])
                        dve.tensor_tensor(
                            out=qsum[jt][:], in0=q2[:, 0:T],
                            in1=q2[:, T:2 * T], op=Alu.add,
                        ).then_inc(s_dve, 1)
                    c += 3; mk("d", "qsum", t, c)
                if 0 <= u:
                    ju = u % 2
                    if dve is not None:
                        dve.wait_ge(s_act, marks[("a", "rcp", u)])
                        if u >= 2:
                            dve.wait_ge(s_out[ju], 16 * (u // 2))
                        dve.tensor_tensor(
                            out=ot_rgb(ju),
                            in0=t3b[ju][:].rearrange("p (c t) -> p c t", c=3),
                            in1=rcp[ju][:].unsqueeze(1).broadcast_to(
                                (P, 3, T)),
                            op=Alu.mult,
                        ).then_inc(s_dve, 1)
                    c += 1; mk("d", "rgb", u, c)

        # pass 1: record marks
        sched_sp(None)
        sched_act(None)
        sched_dve(None)

        blk = ctx.enter_context(nc.Block())

        @blk.sync
        def _(sp):
            sched_sp(sp)

        @blk.scalar
        def _(act):
            sched_act(act)

        @blk.vector
        def _(dve):
            sched_dve(dve)

    return nc


_CACHE = {}


def _get_program():
    if "nc" not in _CACHE:
        _CACHE["nc"] = build_program()
    return _CACHE["nc"]


def _pack_core(zb, ds, pf, pc, bf16_t):
    """Per-core input: [P, NT*IN_W] u16 blob (z|col) and [P, NT*TK] d."""
    mask = pf >= 0
    z_inv = (ZFAR - zb) * (np.float32(1.0) / (ZFAR - ZNEAR))
    z_inv = np.where(mask, z_inv, np.float32(0.0))
    z16 = np.clip(np.rint(z_inv * np.float32(65535.0)), 0, 65535).astype(
        np.uint16
    )
    d_eff = np.where(mask, ds, np.float32(1.0)).astype(bf16_t).view(np.uint16)

    # pixel p-major: (H*W, K[,3]) -> [P, NT, ...] k-major tiles
    z16 = (
        z16.reshape(P, NT, T, K).transpose(0, 1, 3, 2).reshape(P, NT, TK)
    )
    d16 = (
        d_eff.reshape(P, NT, T, K).transpose(0, 1, 3, 2).reshape(P, NT * TK)
    )
    c16 = (
        pc.astype(bf16_t)
        .view(np.uint16)
        .reshape(P, NT, T, K, 3)
        .transpose(0, 1, 4, 3, 2)
        .reshape(P, NT, TK * 3)
    )
    blob = np.ascontiguousarray(
        np.concatenate([z16, c16], axis=2)
    ).reshape(P, NT * IN_W)
    return blob, np.ascontiguousarray(d16)


def _run(pixel_colors, zbuf, dists, pix_to_face, trace=False):
    import ml_dtypes
    from concourse.bass_utils import run_bass_kernel_spmd

    bf16_t = ml_dtypes.bfloat16

    N, H, W, Kk = zbuf.shape
    assert (N, H, W, Kk) == (N_CORES, 512, 512, K), (N, H, W, Kk)

    nc = _get_program()

    pc = np.asarray(pixel_colors, dtype=np.float32)
    zb = np.asarray(zbuf, dtype=np.float32)
    ds = np.asarray(dists, dtype=np.float32)
    pf = np.asarray(pix_to_face)

    in_maps = []
    for i in range(N_CORES):
        blob, din = _pack_core(
            zb[i].reshape(-1, K),
            ds[i].reshape(-1, K),
            pf[i].reshape(-1, K),
            pc[i].reshape(-1, K, 3),
            bf16_t,
        )
        in_maps.append({"inb": blob, "din": din})

    res = run_bass_kernel_spmd(
        nc, in_maps, core_ids=list(range(N_CORES)), trace=trace
    )
    outs = []
    for i in range(N_CORES):
        o = res.results[i]["out"]  # [P, NT*OUT_W] u16
        o = (
            np.ascontiguousarray(o)
            .view(bf16_t)
            .reshape(P, NT, 4, T)
            .transpose(0, 1, 3, 2)
            .astype(np.float32)
            .reshape(H, W, 4)
        )
        outs.append(o)
    return np.stack(outs, axis=0), res


def kernel(pixel_colors, zbuf, dists, pix_to_face):
    out, _ = _run(pixel_colors, zbuf, dists, pix_to_face, trace=False)
    return out
